# revision 14
# baseline (speedup 1.0000x reference)
"""Cost-volume concatenation kernel for Trainium2 (8 NeuronCores).

Reference (per batch b, disparity index d, i = d + MIN_DISP):
  out[b, d, h, w, 0:C]  = left[b, h, w, :]    if 0 <= w - i < W else 0
  out[b, d, h, w, C:2C] = right[b, h, w-i, :] if 0 <= w - i < W else 0

Sharding: disparity-parallel, interleaved -- core c builds disparities
{8j + c : j in 0..15} for the full [B, H, W] volume.  Interleaving
balances valid-span widths (bytes written) across cores.

SPMD trick: run_bass_kernel_spmd runs ONE program on all 8 cores, so the
per-core offset c cannot appear in any access pattern.  The program is
written for i0 = 8j - 112 and all c-dependence lives in the data:
  * rightp input = right pre-shifted by +c columns, zero-padded to W+8
    columns -- the program's static gather rightp[w - i0] then yields
    right[w - i] with the out-of-range mask applied by the padding.
  * cvec input = per-partition scalars [16c, 16(W+c)]; the left-half
    validity mask (left is zeroed outside the valid span) is built
    on-chip: mask[x] = (iota(x) >= 16c) * (iota(x) < 16(W+c)) over
    expanded columns x = 16*w_src + ch.
Each plane writes the union-over-c of valid w-spans; columns inside the
union but outside the core's true span receive exact zeros from the
padding/mask; columns outside the union are never written and rely on
ExternalOutput buffers being pre-zeroed (bass2jax donates zero buffers
to PJRT for exactly this purpose).

Tiles: one disparity plane per SBUF tile, 96 h-rows.  Consecutive planes
are staggered by 32 partitions (even -> rows 0:96, odd -> rows 32:128)
and stored on the two HWDGE rings (sync/scalar): a lone 96-partition DMA
only engages 12 of the 16 SBUF AXI ports (~250 GB/s measured); two
staggered concurrent stores cover all 16 (~330 GB/s measured for 128p).
ScalarE copies the right half, VectorE multiplies the left half by the
mask; one DMA per plane stores the union w-span (0.9-2.3 MB).
"""

import os
import sys

sys.path.insert(0, "/opt/trn_rl_repo")

import numpy as np

B, H, W, C = 2, 96, 192, 16
D = 128
MIN_DISP = -112
N_CORES = 8
DPC = D // N_CORES         # 16 disparity planes per core
PAD = 8                    # rightp padded to W + PAD source columns
WP = W + PAD
COLS = W * 2 * C           # 6144 interleaved f32 per (b,d,h) row

_CACHE = {}


def _plane_span(j):
    """Union-over-c valid w-span for plane j (program-static)."""
    i0 = 8 * j + MIN_DISP
    if i0 < 0:
        us, ue = 0, min(W + i0 + (N_CORES - 1), W)
    else:
        us, ue = i0, W
    return i0, us, ue


def _build_program():
    from concourse import bacc, mybir
    import concourse.tile as tile

    nc = bacc.Bacc(
        "TRN2", target_bir_lowering=False, debug=False, num_devices=N_CORES
    )
    f32 = mybir.dt.float32
    left = nc.dram_tensor("left", [B, H, W * C], f32, kind="ExternalInput")
    rightp = nc.dram_tensor("rightp", [B, H, WP * C], f32, kind="ExternalInput")
    mask = nc.dram_tensor("mask", [128, WP * C], f32, kind="ExternalInput")
    out = nc.dram_tensor("out", [B, DPC, H, COLS], f32, kind="ExternalOutput")

    with tile.TileContext(nc) as tc:
        with (
            tc.tile_pool(name="inputs", bufs=1) as ipool,
            tc.tile_pool(name="work", bufs=4) as wpool,
        ):
            # Input tiles, two stagger phases: phase 0 data at rows 0:96,
            # phase 1 at rows 32:128.
            lsb = {}   # (b, phase) -> (tile, row0)
            rsb = {}
            for b in range(B):
                for ph in range(2):
                    r0 = 32 * ph
                    lt = ipool.tile([128, W * C], f32, tag=f"l{b}{ph}")
                    rt = ipool.tile([128, WP * C], f32, tag=f"r{b}{ph}")
                    lsb[(b, ph)] = (lt, r0)
                    rsb[(b, ph)] = (rt, r0)

            # Host-built mask loaded first on the SWDGE queue so it is
            # ready before the first left-multiply; phase-0 b=0 loads at
            # the heads of the two (empty) HWDGE store rings; everything
            # else behind on the SWDGE queue.
            msk = ipool.tile([128, WP * C], f32, tag="msk")
            nc.gpsimd.dma_start(msk[:, :], mask.ap())
            nc.sync.dma_start(lsb[(0, 0)][0][0:96, :], left.ap()[0])
            nc.scalar.dma_start(rsb[(0, 0)][0][0:96, :], rightp.ap()[0])
            nc.gpsimd.dma_start(lsb[(0, 1)][0][32:128, :], left.ap()[0])
            nc.gpsimd.dma_start(rsb[(0, 1)][0][32:128, :], rightp.ap()[0])
            for b2 in range(1, B):
                nc.gpsimd.dma_start(lsb[(b2, 0)][0][0:96, :], left.ap()[b2])
                nc.gpsimd.dma_start(rsb[(b2, 0)][0][0:96, :], rightp.ap()[b2])
                nc.gpsimd.dma_start(lsb[(b2, 1)][0][32:128, :], left.ap()[b2])
                nc.gpsimd.dma_start(rsb[(b2, 1)][0][32:128, :], rightp.ap()[b2])

            store_engines = [nc.sync, nc.scalar]
            for n in range(B * DPC):
                b, idx = divmod(n, DPC)
                j = (idx + 2) % DPC   # end each batch on the narrowest
                ph = n % 2
                i0, us, ue = _plane_span(j)
                nw = ue - us
                x0 = us - i0      # source column offset into rightp/mask

                lt, r0 = lsb[(b, ph)]
                rt, _ = rsb[(b, ph)]
                T = wpool.tile([128, COLS], f32, tag="out")
                # Compute-engine APs must start in a naturally-aligned
                # partition block, so the 32-offset phase runs one full
                # [0:128) op: rows 0:32 compute garbage from never-
                # written input rows, but are never stored.  Same wall
                # time as a 96-row op (time ~ free size, lanes are
                # parallel), vs 2x for a [32:64)+[64:128) split.
                segs = [(0, 128)] if r0 == 32 else [(0, 96)]
                for s0, sn in segs:
                    s1 = s0 + sn
                    t_chunk = T[s0:s1, us * 32 : ue * 32].rearrange(
                        "p (w c) -> p w c", c=32
                    )
                    src_r = rt[s0:s1, x0 * C : (x0 + nw) * C].rearrange(
                        "p (w c) -> p w c", c=C
                    )
                    src_l = lt[s0:s1, us * C : ue * C].rearrange(
                        "p (w c) -> p w c", c=C
                    )
                    src_m = msk[s0:s1, x0 * C : (x0 + nw) * C].rearrange(
                        "p (w c) -> p w c", c=C
                    )
                    nc.scalar.copy(t_chunk[:, :, C : 2 * C], src_r)
                    nc.vector.tensor_mul(t_chunk[:, :, 0:C], src_l, src_m)

                dst = out.ap()[b, j, :, us * 32 : ue * 32]
                store_engines[ph].dma_start(
                    dst, T[r0 : r0 + H, us * 32 : ue * 32]
                )

    nc.compile()
    return nc


def _get_program():
    if "nc" not in _CACHE:
        _CACHE["nc"] = _build_program()
    return _CACHE["nc"]


def kernel(left, right):
    from concourse.bass_utils import run_bass_kernel_spmd

    left = np.ascontiguousarray(left, dtype=np.float32).reshape(B, H, W * C)
    right = np.ascontiguousarray(right, dtype=np.float32)
    nc = _get_program()

    in_maps = []
    for c in range(N_CORES):
        rp = np.zeros((B, H, WP, C), dtype=np.float32)
        rp[:, :, c : c + W] = right
        m = np.zeros(WP * C, dtype=np.float32)
        m[C * c : C * (W + c)] = 1.0
        in_maps.append(
            {
                "left": left,
                "rightp": rp.reshape(B, H, WP * C),
                "mask": np.ascontiguousarray(
                    np.broadcast_to(m, (128, WP * C))
                ),
            }
        )

    prof_dir = os.environ.get("BASS_NTFF_DIR")
    if prof_dir:
        from trn_agent_boot.trn_boot import _ntff_profile_via_ctypes

        hook = _ntff_profile_via_ctypes("/opt/axon/libaxon_pjrt.so")
        with hook(prof_dir, [0]):
            res = run_bass_kernel_spmd(nc, in_maps, core_ids=list(range(N_CORES)))
    else:
        res = run_bass_kernel_spmd(nc, in_maps, core_ids=list(range(N_CORES)))

    # parts[c][b, j] is disparity d = 8j + c -> stack on a new axis after j.
    parts = [
        res.results[c]["out"].reshape(B, DPC, H, W, 2 * C)
        for c in range(N_CORES)
    ]
    return np.stack(parts, axis=2).reshape(B, D, H, W, 2 * C)


# revision 15
# speedup vs baseline: 1.0205x; 1.0205x over previous
"""Cost-volume concatenation kernel for Trainium2 (8 NeuronCores).

Reference (per batch b, disparity index d, i = d + MIN_DISP):
  out[b, d, h, w, 0:C]  = left[b, h, w, :]    if 0 <= w - i < W else 0
  out[b, d, h, w, C:2C] = right[b, h, w-i, :] if 0 <= w - i < W else 0

Sharding: disparity-parallel, interleaved -- core c builds disparities
{8j + c : j in 0..15} for the full [B, H, W] volume.  Interleaving
balances valid-span widths (bytes written) across cores.

SPMD trick: run_bass_kernel_spmd runs ONE program on all 8 cores, so the
per-core offset c cannot appear in any access pattern.  The program is
written for i0 = 8j - 112 and all c-dependence lives in the data:
  * rightp input = right pre-shifted by +c columns, zero-padded to W+8
    columns -- the program's static gather rightp[w - i0] then yields
    right[w - i] with the out-of-range mask applied by the padding.
  * cvec input = per-partition scalars [16c, 16(W+c)]; the left-half
    validity mask (left is zeroed outside the valid span) is built
    on-chip: mask[x] = (iota(x) >= 16c) * (iota(x) < 16(W+c)) over
    expanded columns x = 16*w_src + ch.
Each plane writes the union-over-c of valid w-spans; columns inside the
union but outside the core's true span receive exact zeros from the
padding/mask; columns outside the union are never written and rely on
ExternalOutput buffers being pre-zeroed (bass2jax donates zero buffers
to PJRT for exactly this purpose).

Tiles: one disparity plane per SBUF tile, 96 h-rows.  Consecutive planes
are staggered by 32 partitions (even -> rows 0:96, odd -> rows 32:128)
and stored on the two HWDGE rings (sync/scalar): a lone 96-partition DMA
only engages 12 of the 16 SBUF AXI ports (~250 GB/s measured); two
staggered concurrent stores cover all 16 (~330 GB/s measured for 128p).
ScalarE copies the right half, VectorE multiplies the left half by the
mask; one DMA per plane stores the union w-span (0.9-2.3 MB).
"""

import os
import sys

sys.path.insert(0, "/opt/trn_rl_repo")

import numpy as np

B, H, W, C = 2, 96, 192, 16
D = 128
MIN_DISP = -112
N_CORES = 8
DPC = D // N_CORES         # 16 disparity planes per core
PAD = 8                    # rightp padded to W + PAD source columns
WP = W + PAD
COLS = W * 2 * C           # 6144 interleaved f32 per (b,d,h) row

_CACHE = {}


def _plane_span(j):
    """Union-over-c valid w-span for plane j (program-static)."""
    i0 = 8 * j + MIN_DISP
    if i0 < 0:
        us, ue = 0, min(W + i0 + (N_CORES - 1), W)
    else:
        us, ue = i0, W
    return i0, us, ue


def _build_program():
    from concourse import bacc, mybir
    import concourse.tile as tile

    nc = bacc.Bacc(
        "TRN2", target_bir_lowering=False, debug=False, num_devices=N_CORES
    )
    f32 = mybir.dt.float32
    left = nc.dram_tensor("left", [B, H, W * C], f32, kind="ExternalInput")
    rightp = nc.dram_tensor("rightp", [B, H, WP * C], f32, kind="ExternalInput")
    mask = nc.dram_tensor("mask", [128, WP * C], f32, kind="ExternalInput")
    out = nc.dram_tensor("out", [B, DPC, H, COLS], f32, kind="ExternalOutput")

    with tile.TileContext(nc) as tc:
        with (
            tc.tile_pool(name="inputs", bufs=1) as ipool,
            tc.tile_pool(name="work", bufs=4) as wpool,
        ):
            # Input tiles, two stagger phases: phase 0 data at rows 0:96,
            # phase 1 at rows 32:128.
            lsb = {}   # (b, phase) -> (tile, row0)
            rsb = {}
            for b in range(B):
                for ph in range(2):
                    r0 = 32 * ph
                    lt = ipool.tile([128, W * C], f32, tag=f"l{b}{ph}")
                    rt = ipool.tile([128, WP * C], f32, tag=f"r{b}{ph}")
                    lsb[(b, ph)] = (lt, r0)
                    rsb[(b, ph)] = (rt, r0)

            # Host-built mask loaded first on the SWDGE queue so it is
            # ready before the first left-multiply; phase-0 b=0 loads at
            # the heads of the two (empty) HWDGE store rings; everything
            # else behind on the SWDGE queue.
            msk = ipool.tile([128, WP * C], f32, tag="msk")
            nc.gpsimd.dma_start(msk[:, :], mask.ap())
            nc.sync.dma_start(lsb[(0, 0)][0][0:96, :], left.ap()[0])
            nc.scalar.dma_start(rsb[(0, 0)][0][0:96, :], rightp.ap()[0])
            nc.gpsimd.dma_start(lsb[(0, 1)][0][32:128, :], left.ap()[0])
            nc.gpsimd.dma_start(rsb[(0, 1)][0][32:128, :], rightp.ap()[0])
            for b2 in range(1, B):
                nc.gpsimd.dma_start(lsb[(b2, 0)][0][0:96, :], left.ap()[b2])
                nc.gpsimd.dma_start(rsb[(b2, 0)][0][0:96, :], rightp.ap()[b2])
                nc.gpsimd.dma_start(lsb[(b2, 1)][0][32:128, :], left.ap()[b2])
                nc.gpsimd.dma_start(rsb[(b2, 1)][0][32:128, :], rightp.ap()[b2])

            store_engines = [nc.sync, nc.scalar]
            for n in range(B * DPC):
                b, j = divmod(n, DPC)
                ph = n % 2
                i0, us, ue = _plane_span(j)
                nw = ue - us
                x0 = us - i0      # source column offset into rightp/mask

                lt, r0 = lsb[(b, ph)]
                rt, _ = rsb[(b, ph)]
                T = wpool.tile([128, COLS], f32, tag="out")
                # Compute-engine APs must start in a naturally-aligned
                # partition block, so the 32-offset phase runs one full
                # [0:128) op: rows 0:32 compute garbage from never-
                # written input rows, but are never stored.  Same wall
                # time as a 96-row op (time ~ free size, lanes are
                # parallel), vs 2x for a [32:64)+[64:128) split.
                segs = [(0, 128)] if r0 == 32 else [(0, 96)]
                for s0, sn in segs:
                    s1 = s0 + sn
                    t_chunk = T[s0:s1, us * 32 : ue * 32].rearrange(
                        "p (w c) -> p w c", c=32
                    )
                    src_r = rt[s0:s1, x0 * C : (x0 + nw) * C].rearrange(
                        "p (w c) -> p w c", c=C
                    )
                    src_l = lt[s0:s1, us * C : ue * C].rearrange(
                        "p (w c) -> p w c", c=C
                    )
                    src_m = msk[s0:s1, x0 * C : (x0 + nw) * C].rearrange(
                        "p (w c) -> p w c", c=C
                    )
                    nc.scalar.copy(t_chunk[:, :, C : 2 * C], src_r)
                    nc.vector.tensor_mul(t_chunk[:, :, 0:C], src_l, src_m)

                dst = out.ap()[b, j, :, us * 32 : ue * 32]
                store_engines[ph].dma_start(
                    dst, T[r0 : r0 + H, us * 32 : ue * 32]
                )

    nc.compile()
    return nc


def _get_program():
    if "nc" not in _CACHE:
        _CACHE["nc"] = _build_program()
    return _CACHE["nc"]


def kernel(left, right):
    from concourse.bass_utils import run_bass_kernel_spmd

    left = np.ascontiguousarray(left, dtype=np.float32).reshape(B, H, W * C)
    right = np.ascontiguousarray(right, dtype=np.float32)
    nc = _get_program()

    in_maps = []
    for c in range(N_CORES):
        rp = np.zeros((B, H, WP, C), dtype=np.float32)
        rp[:, :, c : c + W] = right
        m = np.zeros(WP * C, dtype=np.float32)
        m[C * c : C * (W + c)] = 1.0
        in_maps.append(
            {
                "left": left,
                "rightp": rp.reshape(B, H, WP * C),
                "mask": np.ascontiguousarray(
                    np.broadcast_to(m, (128, WP * C))
                ),
            }
        )

    prof_dir = os.environ.get("BASS_NTFF_DIR")
    if prof_dir:
        from trn_agent_boot.trn_boot import _ntff_profile_via_ctypes

        hook = _ntff_profile_via_ctypes("/opt/axon/libaxon_pjrt.so")
        with hook(prof_dir, [0]):
            res = run_bass_kernel_spmd(nc, in_maps, core_ids=list(range(N_CORES)))
    else:
        res = run_bass_kernel_spmd(nc, in_maps, core_ids=list(range(N_CORES)))

    # parts[c][b, j] is disparity d = 8j + c -> stack on a new axis after j.
    parts = [
        res.results[c]["out"].reshape(B, DPC, H, W, 2 * C)
        for c in range(N_CORES)
    ]
    return np.stack(parts, axis=2).reshape(B, D, H, W, 2 * C)


# revision 16
# speedup vs baseline: 1.0252x; 1.0046x over previous
"""Cost-volume concatenation kernel for Trainium2 (8 NeuronCores).

Reference (per batch b, disparity index d, i = d + MIN_DISP):
  out[b, d, h, w, 0:C]  = left[b, h, w, :]    if 0 <= w - i < W else 0
  out[b, d, h, w, C:2C] = right[b, h, w-i, :] if 0 <= w - i < W else 0

Sharding: disparity-parallel, interleaved -- core c builds disparities
{8j + c : j in 0..15} for the full [B, H, W] volume.  Interleaving
balances valid-span widths (bytes written) across cores.

SPMD trick: run_bass_kernel_spmd runs ONE program on all 8 cores, so the
per-core offset c cannot appear in any access pattern.  The program is
written for i0 = 8j - 112 and all c-dependence lives in the data:
  * rightp input = right pre-shifted by +c columns, zero-padded to W+8
    columns -- the program's static gather rightp[w - i0] then yields
    right[w - i] with the out-of-range mask applied by the padding.
  * cvec input = per-partition scalars [16c, 16(W+c)]; the left-half
    validity mask (left is zeroed outside the valid span) is built
    on-chip: mask[x] = (iota(x) >= 16c) * (iota(x) < 16(W+c)) over
    expanded columns x = 16*w_src + ch.
Each plane writes the union-over-c of valid w-spans; columns inside the
union but outside the core's true span receive exact zeros from the
padding/mask; columns outside the union are never written and rely on
ExternalOutput buffers being pre-zeroed (bass2jax donates zero buffers
to PJRT for exactly this purpose).

Tiles: one disparity plane per SBUF tile, 96 h-rows.  Consecutive planes
are staggered by 32 partitions (even -> rows 0:96, odd -> rows 32:128)
and stored on the two HWDGE rings (sync/scalar): a lone 96-partition DMA
only engages 12 of the 16 SBUF AXI ports (~250 GB/s measured); two
staggered concurrent stores cover all 16 (~330 GB/s measured for 128p).
ScalarE copies the right half, VectorE multiplies the left half by the
mask; one DMA per plane stores the union w-span (0.9-2.3 MB).
"""

import os
import sys

sys.path.insert(0, "/opt/trn_rl_repo")

import numpy as np

B, H, W, C = 2, 96, 192, 16
D = 128
MIN_DISP = -112
N_CORES = 8
DPC = D // N_CORES         # 16 disparity planes per core
PAD = 8                    # rightp padded to W + PAD source columns
WP = W + PAD
COLS = W * 2 * C           # 6144 interleaved f32 per (b,d,h) row

_CACHE = {}


def _plane_span(j):
    """Union-over-c valid w-span for plane j (program-static)."""
    i0 = 8 * j + MIN_DISP
    if i0 < 0:
        us, ue = 0, min(W + i0 + (N_CORES - 1), W)
    else:
        us, ue = i0, W
    return i0, us, ue


def _build_program():
    from concourse import bacc, mybir
    import concourse.tile as tile

    nc = bacc.Bacc(
        "TRN2", target_bir_lowering=False, debug=False, num_devices=N_CORES
    )
    f32 = mybir.dt.float32
    left = nc.dram_tensor("left", [B, H, W * C], f32, kind="ExternalInput")
    rightp = nc.dram_tensor("rightp", [B, H, WP * C], f32, kind="ExternalInput")
    cvec = nc.dram_tensor("cvec", [128, 2], f32, kind="ExternalInput")
    out = nc.dram_tensor("out", [B, DPC, H, COLS], f32, kind="ExternalOutput")

    with tile.TileContext(nc) as tc:
        with (
            tc.tile_pool(name="inputs", bufs=1) as ipool,
            tc.tile_pool(name="work", bufs=4) as wpool,
        ):
            # Input tiles, two stagger phases: phase 0 data at rows 0:96,
            # phase 1 at rows 32:128.
            lsb = {}   # (b, phase) -> (tile, row0)
            rsb = {}
            for b in range(B):
                for ph in range(2):
                    r0 = 32 * ph
                    lt = ipool.tile([128, W * C], f32, tag=f"l{b}{ph}")
                    rt = ipool.tile([128, WP * C], f32, tag=f"r{b}{ph}")
                    lsb[(b, ph)] = (lt, r0)
                    rsb[(b, ph)] = (rt, r0)

            # Tiny cvec load + iota first so the mask is ready early;
            # phase-0 b=0 loads at the heads of the two (empty) HWDGE
            # store rings; everything else behind on the SWDGE queue.
            cv = ipool.tile([128, 2], f32, tag="cvec")
            nc.gpsimd.dma_start(cv[:, :], cvec.ap())
            # xio borrows a work-pool slot; it is dead after the mask
            # build and the slot returns to the store pipeline.
            xio = wpool.tile([128, COLS], f32, tag="out")
            msk = ipool.tile([128, WP * C], f32, tag="msk")
            nc.gpsimd.iota(
                xio[:, 0:WP * C], [[1, WP * C]], channel_multiplier=0,
                allow_small_or_imprecise_dtypes=True,
            )
            nc.sync.dma_start(lsb[(0, 0)][0][0:96, :], left.ap()[0])
            nc.scalar.dma_start(rsb[(0, 0)][0][0:96, :], rightp.ap()[0])
            nc.gpsimd.dma_start(lsb[(0, 1)][0][32:128, :], left.ap()[0])
            nc.gpsimd.dma_start(rsb[(0, 1)][0][32:128, :], rightp.ap()[0])
            for b2 in range(1, B):
                nc.gpsimd.dma_start(lsb[(b2, 0)][0][0:96, :], left.ap()[b2])
                nc.gpsimd.dma_start(rsb[(b2, 0)][0][0:96, :], rightp.ap()[b2])
                nc.gpsimd.dma_start(lsb[(b2, 1)][0][32:128, :], left.ap()[b2])
                nc.gpsimd.dma_start(rsb[(b2, 1)][0][32:128, :], rightp.ap()[b2])

            # Mask over expanded source columns x = 16*w_src + ch,
            # identical on every partition: 1.0 iff 16c <= x < 16(W+c).
            nc.vector.tensor_single_scalar(
                msk[:, :], xio[:, 0:WP * C], cv[:, 0:1], mybir.AluOpType.is_ge
            )
            nc.vector.tensor_single_scalar(
                xio[:, 0:WP * C], xio[:, 0:WP * C], cv[:, 1:2],
                mybir.AluOpType.is_lt
            )
            nc.vector.tensor_mul(msk[:, :], msk[:, :], xio[:, 0:WP * C])

            store_engines = [nc.sync, nc.scalar]
            for n in range(B * DPC):
                b, j = divmod(n, DPC)
                ph = n % 2
                i0, us, ue = _plane_span(j)
                nw = ue - us
                x0 = us - i0      # source column offset into rightp/mask

                lt, r0 = lsb[(b, ph)]
                rt, _ = rsb[(b, ph)]
                T = wpool.tile([128, COLS], f32, tag="out")
                # Compute-engine APs must start in a naturally-aligned
                # partition block, so the 32-offset phase runs one full
                # [0:128) op: rows 0:32 compute garbage from never-
                # written input rows, but are never stored.  Same wall
                # time as a 96-row op (time ~ free size, lanes are
                # parallel), vs 2x for a [32:64)+[64:128) split.
                segs = [(0, 128)] if r0 == 32 else [(0, 96)]
                for s0, sn in segs:
                    s1 = s0 + sn
                    t_chunk = T[s0:s1, us * 32 : ue * 32].rearrange(
                        "p (w c) -> p w c", c=32
                    )
                    src_r = rt[s0:s1, x0 * C : (x0 + nw) * C].rearrange(
                        "p (w c) -> p w c", c=C
                    )
                    src_l = lt[s0:s1, us * C : ue * C].rearrange(
                        "p (w c) -> p w c", c=C
                    )
                    src_m = msk[s0:s1, x0 * C : (x0 + nw) * C].rearrange(
                        "p (w c) -> p w c", c=C
                    )
                    nc.scalar.copy(t_chunk[:, :, C : 2 * C], src_r)
                    nc.vector.tensor_mul(t_chunk[:, :, 0:C], src_l, src_m)

                dst = out.ap()[b, j, :, us * 32 : ue * 32]
                store_engines[ph].dma_start(
                    dst, T[r0 : r0 + H, us * 32 : ue * 32]
                )

    nc.compile()
    return nc


def _get_program():
    if "nc" not in _CACHE:
        _CACHE["nc"] = _build_program()
    return _CACHE["nc"]


def kernel(left, right):
    from concourse.bass_utils import run_bass_kernel_spmd

    left = np.ascontiguousarray(left, dtype=np.float32).reshape(B, H, W * C)
    right = np.ascontiguousarray(right, dtype=np.float32)
    nc = _get_program()

    in_maps = []
    for c in range(N_CORES):
        rp = np.zeros((B, H, WP, C), dtype=np.float32)
        rp[:, :, c : c + W] = right
        cv = np.empty((128, 2), dtype=np.float32)
        cv[:, 0] = 16.0 * c
        cv[:, 1] = 16.0 * (W + c)
        in_maps.append(
            {
                "left": left,
                "rightp": rp.reshape(B, H, WP * C),
                "cvec": cv,
            }
        )

    prof_dir = os.environ.get("BASS_NTFF_DIR")
    if prof_dir:
        from trn_agent_boot.trn_boot import _ntff_profile_via_ctypes

        hook = _ntff_profile_via_ctypes("/opt/axon/libaxon_pjrt.so")
        with hook(prof_dir, [0]):
            res = run_bass_kernel_spmd(nc, in_maps, core_ids=list(range(N_CORES)))
    else:
        res = run_bass_kernel_spmd(nc, in_maps, core_ids=list(range(N_CORES)))

    # parts[c][b, j] is disparity d = 8j + c -> stack on a new axis after j.
    parts = [
        res.results[c]["out"].reshape(B, DPC, H, W, 2 * C)
        for c in range(N_CORES)
    ]
    return np.stack(parts, axis=2).reshape(B, D, H, W, 2 * C)
